# revision 33
# baseline (speedup 1.0000x reference)
"""Trainium2 Bass kernel for a text-adapter block (LN -> 768->16 -> ReLU ->
16->768 -> *0.1 -> +residual), data-parallel over 8 NeuronCores.

Self-contained: takes the FULL inputs from setup_inputs(), shards x on the
token axis across 8 cores, runs one compiled Bass module SPMD, gathers.

I/O is bf16 (host converts f32<->bf16): the kernel is HBM-bound and the
2e-2 relative tolerance comfortably absorbs bf16 rounding of x and out.

Layout: the per-core [4096, 768] x is viewed as [128, 32, 768] -- SBUF
partition p holds tokens p*32..p*32+31 -- so every DMA moves one fully
contiguous multi-KB segment per partition (minimal descriptor count on
the SP queue).  A "tile" j is the 128 tokens {p*32+j}; LN is per-token
so the interleaving is math-neutral and host gather is a plain reshape.

Engine balance per tile (sim cost model):
  DVE : bn_stats x2, bn_aggr, rsqrt chain, tiny stat copies  (~1.1us)
  ACT : sqrt, relu, single 768-col x^T evac PSUM->SBUF       (~0.9us)
  Pool: single fused final combine out = pup*rstd + x (stt)  (~0.8us)
  PE  : 7 transposes + 7 down + 2 up + 1 bias matmul         (~1.2us)
  SP  : 1 quad DMA in + 1 quad DMA out per 4 tiles           (~0.3us)

Math (exact, with LN affine folded into the adapter weights on the host):
  W' = gamma[:,None] * w_down          b' = b_down + beta @ w_down
  c  = colsum(W')                      psi[k,t] = sum_d x[d,t] W'[d,k]
                                                + std_t * b'[k] - mean_t * c[k]
  down[k,t] = rstd_t * relu(psi[k,t])          (relu commutes with rstd>0)
  pup[t,d]  = sum_k relu(psi)[k,t] * (0.1*w_up)[k,d] + std_t * (0.1*b_up)[d]
              (bias via a concurrent K=1 matmul from row-group 32; std*rstd=1)
  out[t,d]  = rstd_t * pup[t,d] + x[t,d]
"""

import numpy as np

D_MODEL = 768
BOTTLENECK = 16
K17 = BOTTLENECK + 1
SCALE = 0.1
LN_EPS = 1e-5
P = 128
N_CORES = 8
CHUNKS = D_MODEL // P  # 6

# x^T PSUM->SBUF evac split: first XT_ACT_CHUNKS by ACT, rest by DVE
# (Pool/GPSIMD cannot access PSUM).
XT_ACT_CHUNKS = 4

# bn_stats split point (max 512 per op)
BN_SPLIT = 512

# Columns of the final (pup*rstd + x) done by DVE stt straight from PSUM;
# the rest is evacuated by ACT (Copy*rstd) and residual-added on Pool.
FINAL_DVE_COLS = 192

# Tiles loaded/stored per DMA instruction (contiguous per partition).
GROUP = 4

# PSUM pool buffer counts (8 banks: XT_BUFS*1 + PSI_BUFS*1 + UP_BUFS*2).
XT_BUFS = 2
PSI_BUFS = 2
UP_BUFS = 2

# Software-pipeline skew: stage_a (load/stats/transpose) emitted SKEW tiles
# ahead of stage_b (down matmuls/relu); stage_c (up/final/store) trails
# stage_b by SKEW_C more tiles so PE never head-of-line-waits on relu.
SKEW = 0
SKEW_C = 1

_CACHE: dict = {}


def _build(rows_per_core: int, reps: int = 1):
    from contextlib import ExitStack

    import concourse.bacc as bacc
    import concourse.tile as tile
    from concourse import mybir

    nc = bacc.Bacc(
        "TRN2",
        target_bir_lowering=False,
        debug=False,
        enable_asserts=False,
        num_devices=N_CORES,
    )
    f32 = mybir.dt.float32
    bf16 = mybir.dt.bfloat16

    ntiles = rows_per_core // P
    x_d = nc.dram_tensor(
        "x", [P, ntiles, D_MODEL], bf16, kind="ExternalInput"
    ).ap()
    wd_d = nc.dram_tensor("wd", [D_MODEL, K17], bf16, kind="ExternalInput").ap()
    cb_d = nc.dram_tensor("cb", [2, K17], bf16, kind="ExternalInput").ap()
    wu_d = nc.dram_tensor("wu", [K17, D_MODEL], bf16, kind="ExternalInput").ap()
    ident_d = nc.dram_tensor("ident", [P, P], bf16, kind="ExternalInput").ap()
    out_d = nc.dram_tensor(
        "out", [P, ntiles, D_MODEL], bf16, kind="ExternalOutput"
    ).ap()

    Relu = mybir.ActivationFunctionType.Relu
    Sqrt = mybir.ActivationFunctionType.Sqrt
    Copy = mybir.ActivationFunctionType.Copy
    mult = mybir.AluOpType.mult
    add = mybir.AluOpType.add

    with tile.TileContext(nc) as tc, ExitStack() as ctx:
        consts = ctx.enter_context(tc.tile_pool(name="consts", bufs=1))
        xpool = ctx.enter_context(tc.tile_pool(name="xpool", bufs=3))
        opool = ctx.enter_context(tc.tile_pool(name="opool", bufs=3))
        xtpool = ctx.enter_context(tc.tile_pool(name="xtpool", bufs=4))
        uppool = ctx.enter_context(tc.tile_pool(name="uppool", bufs=2))
        tiny = ctx.enter_context(tc.tile_pool(name="tiny", bufs=8))
        p_xt = ctx.enter_context(tc.tile_pool(name="p_xt", bufs=XT_BUFS, space="PSUM"))
        p_psi = ctx.enter_context(tc.tile_pool(name="p_psi", bufs=PSI_BUFS, space="PSUM"))
        p_up = ctx.enter_context(tc.tile_pool(name="p_up", bufs=UP_BUFS, space="PSUM"))

        # ---- constants (loaded once; ident first, stage_a needs it) ----
        ident_sb = consts.tile([P, P], bf16)
        nc.sync.dma_start(out=ident_sb, in_=ident_d)
        eps_sb = consts.tile([P, 1], f32)
        nc.vector.memset(eps_sb, LN_EPS)
        wd_sb = consts.tile([P, CHUNKS, K17], bf16)
        cb_sb = consts.tile([2, K17], bf16)
        wu_sb = consts.tile([K17, D_MODEL], bf16)

        def load_b_consts():
            nc.sync.dma_start(
                out=wd_sb, in_=wd_d.rearrange("(c p) k -> p c k", p=P)
            )
            nc.sync.dma_start(out=cb_sb, in_=cb_d)

        def load_c_consts():
            nc.sync.dma_start(out=wu_sb, in_=wu_d)

        group_x: dict = {}

        def stage_a(i):
            """DMA-in, LN stats chain, PE transposes, ACT evac."""
            j = i % ntiles
            g = i % GROUP
            if g == 0:
                xq = xpool.tile([P, GROUP, D_MODEL], bf16, tag="x")
                nc.sync.dma_start(out=xq, in_=x_d[:, j : j + GROUP, :])
                group_x[i] = xq
            else:
                xq = group_x[i - g]
                if g == GROUP - 1:
                    del group_x[i - g]
            x_sb = xq[:, g, :]

            st6 = tiny.tile([P, 2, 6], f32, tag="st6")
            nc.vector.bn_stats(out=st6[:, 0, :], in_=x_sb[:, 0:BN_SPLIT])
            nc.vector.bn_stats(out=st6[:, 1, :], in_=x_sb[:, BN_SPLIT:])
            mv = tiny.tile([P, 2], f32, tag="mv")  # (mean, var)
            nc.vector.bn_aggr(out=mv, in_=st6)
            # mvb columns are (std, mean): std lands in partition 0 of the
            # transposed stats row, where 32-aligned AP reads can reach it.
            # ACT sqrt writes bf16 std straight into mvb (one hop shorter);
            # rstd derives from the bf16 std (0.4% rounding, well in tol).
            mvb = tiny.tile([P, 2], bf16, tag="mvb")
            nc.vector.tensor_copy(out=mvb[:, 1:2], in_=mv[:, 0:1])
            nc.scalar.activation(
                out=mvb[:, 0:1], in_=mv[:, 1:2], func=Sqrt, bias=eps_sb
            )
            rstd = tiny.tile([P, 1], f32, tag="rstd")
            nc.vector.reciprocal(out=rstd, in_=mvb[:, 0:1])

            # pxt also hosts the [2, P] stats transpose at cols 768:896
            # (all writes are full-overwrite start=True: no accumulation,
            # so sharing a bank is hazard-free).
            pxt = p_xt.tile([P, D_MODEL + P], bf16, tag="pxt")
            # stats transpose first: frees the DVE msrow evac to run early
            nc.tensor.transpose(
                out=pxt[0:2, D_MODEL : D_MODEL + P], in_=mvb, identity=ident_sb
            )
            for c in range(CHUNKS):
                nc.tensor.transpose(
                    out=pxt[:, c * P : (c + 1) * P],
                    in_=x_sb[:, c * P : (c + 1) * P],
                    identity=ident_sb,
                )
            msrow = tiny.tile([2, P], bf16, tag="msrow")  # rows (std, mean)
            nc.vector.tensor_copy(out=msrow, in_=pxt[0:2, D_MODEL : D_MODEL + P])
            CA = XT_ACT_CHUNKS
            xt_b16 = xtpool.tile([P, D_MODEL], bf16, tag="xt")
            nc.scalar.activation(
                out=xt_b16[:, 0 : CA * P], in_=pxt[:, 0 : CA * P], func=Copy
            )
            if CA < CHUNKS:
                nc.vector.tensor_copy(
                    out=xt_b16[:, CA * P :], in_=pxt[:, CA * P : D_MODEL]
                )
            return dict(x_sb=x_sb, rstd=rstd, msrow=msrow, xt_b16=xt_b16, j=j)

        def stage_b(s):
            """down-proj matmuls + relu evac.

            Row 16 of ppsi carries std_t (wd col 16 is zero, cb col 16 is
            (1,0) against the (std, mean) rows): relu(std)=std since std>0,
            so the K=17 up matmul applies the up bias as std*0.1*b_up and
            the final *rstd makes it exactly 0.1*b_up.
            """
            ppsi = p_psi.tile([K17, P], f32, tag="psi")
            for c in range(CHUNKS):
                nc.tensor.matmul(
                    ppsi,
                    lhsT=wd_sb[:, c, :],
                    rhs=s["xt_b16"][:, c * P : (c + 1) * P],
                    start=(c == 0),
                    stop=False,
                )
            # corrections: b' (x) std - c (x) mean   (K=2 rank-2 update)
            nc.tensor.matmul(
                ppsi, lhsT=cb_sb, rhs=s["msrow"], start=False, stop=True
            )
            relu17 = tiny.tile([K17, P], bf16, tag="relu")
            nc.scalar.activation(out=relu17, in_=ppsi, func=Relu)
            s["relu17"] = relu17

        group_o: dict = {}

        def stage_c(s, i):
            """up-proj + concurrent bias matmul, fused final combine, DMA-out."""
            pup = p_up.tile([P, D_MODEL], f32, tag="pup")
            for lo, hi in ((0, 512), (512, D_MODEL)):
                nc.tensor.matmul(
                    pup[:, lo:hi],
                    lhsT=s["relu17"],
                    rhs=wu_sb[:, lo:hi],
                    start=True,
                    stop=True,
                )
            j = s["j"]
            g = i % GROUP
            if g == 0:
                group_o[i] = opool.tile(
                    [P, GROUP, D_MODEL], bf16, tag="out", name=f"ogrp{i}"
                )
            oq = group_o[i - g]
            S = FINAL_DVE_COLS
            if S > 0:
                nc.vector.scalar_tensor_tensor(
                    out=oq[:, g, 0:S],
                    in0=pup[:, 0:S],
                    scalar=s["rstd"],
                    in1=s["x_sb"][:, 0:S],
                    op0=mult,
                    op1=add,
                )
            if S < D_MODEL:
                up_sb = uppool.tile([P, D_MODEL - S], bf16, tag="up")
                nc.scalar.activation(
                    out=up_sb, in_=pup[:, S:], func=Copy, scale=s["rstd"]
                )
                nc.gpsimd.tensor_add(oq[:, g, S:], up_sb, s["x_sb"][:, S:])
            if g == GROUP - 1:
                nc.sync.dma_start(
                    out=out_d[:, j - g : j + 1, :], in_=group_o.pop(i - g)
                )

        # software-pipelined emission: stage_a runs SKEW tiles ahead of
        # stage_b, and stage_c trails stage_b by SKEW_C tiles, so each
        # engine's FIFO always has independent work before an instruction
        # that waits on another engine's result.
        n = ntiles * reps
        live_a: list = []
        live_b: list = []
        n_c = 0
        for i in range(n + SKEW + SKEW_C):
            if i < n:
                live_a.append(stage_a(i))
            if i == 0:
                load_b_consts()
            if i == 1:
                load_c_consts()
            if i >= SKEW and live_a:
                s = live_a.pop(0)
                stage_b(s)
                live_b.append(s)
            if i >= SKEW + SKEW_C and live_b:
                stage_c(live_b.pop(0), n_c)
                n_c += 1

    nc.compile()
    return nc


def _get_nc(rows_per_core: int, reps: int = 1):
    key = (rows_per_core, reps)
    if key not in _CACHE:
        _CACHE[key] = _build(rows_per_core, reps)
    return _CACHE[key]


def _host_consts(ln_gamma, ln_beta, w_down, b_down, w_up, b_up):
    import ml_dtypes

    bf = ml_dtypes.bfloat16
    ln_gamma = np.asarray(ln_gamma, np.float32)
    ln_beta = np.asarray(ln_beta, np.float32)
    w_down = np.asarray(w_down, np.float32)
    b_down = np.asarray(b_down, np.float32)
    w_up = np.asarray(w_up, np.float32)
    b_up = np.asarray(b_up, np.float32)

    wd_eff = ln_gamma[:, None] * w_down
    b_eff = b_down + ln_beta @ w_down
    # column 16 routes std through psi row 16 (wd col 16 = 0; cb col 16 =
    # (1, 0) against the (std, mean) stat rows); wu row 16 is the up bias.
    wd17 = np.concatenate(
        [wd_eff, np.zeros((D_MODEL, 1), np.float32)], axis=1
    )
    cb17 = np.stack(
        [
            np.concatenate([b_eff, [1.0]]),
            np.concatenate([-wd_eff.sum(0), [0.0]]),
        ]
    )
    wu17 = np.concatenate([SCALE * w_up, (SCALE * b_up)[None, :]], axis=0)
    return dict(
        wd=wd17.astype(bf),
        cb=np.ascontiguousarray(cb17).astype(bf),
        wu=wu17.astype(bf),
        ident=np.eye(P, dtype=np.float32).astype(bf),
    )


def kernel(x, ln_gamma, ln_beta, w_down, b_down, w_up, b_up):
    import ml_dtypes
    from concourse.bass_utils import run_bass_kernel_spmd

    bf = ml_dtypes.bfloat16
    x = np.asarray(x)
    b, t, d = x.shape
    rows = b * t
    rpc = rows // N_CORES
    ntiles = rpc // P
    consts = _host_consts(ln_gamma, ln_beta, w_down, b_down, w_up, b_up)
    xf = np.ascontiguousarray(x.reshape(rows, d)).astype(bf)
    in_maps = [
        dict(
            x=xf[i * rpc : (i + 1) * rpc].reshape(P, ntiles, d),
            **consts,
        )
        for i in range(N_CORES)
    ]
    nc = _get_nc(rpc)
    res = run_bass_kernel_spmd(nc, in_maps, core_ids=list(range(N_CORES)))
    out = np.concatenate(
        [r["out"].reshape(rpc, d) for r in res.results], axis=0
    )
    return np.ascontiguousarray(out.reshape(b, t, d).astype(np.float32))
